# revision 76
# baseline (speedup 1.0000x reference)
"""Transformer-XL relative multi-head attention, 8-way sharded on Trainium2.

Self-contained harness entry: kernel(**inputs) -> np.ndarray [4, 1024, 1024].

Sharding: core c handles batch b = c//2 and head-half hh = c%2 (8 of 16
heads). Each core computes a partial output (its heads' contribution
through Wo); the host unshard sums the two partials per batch (row-parallel
tensor parallelism for the output projection).

Pipeline, software-pipelined LA=4 iterations deep over (head-pair hp,
query tile qi), both heads h chunk-interleaved so K=64 matmuls pack in
PE row groups:
  - m_stage: position matmuls M_h [128, W] -> PSUM; evacuation to fp16
    fused with the causal mask (DVE scalar_tensor_tensor adding the
    precomputed mm2 plane; unmasked chunks split DVE/ACT); shear write
    to a DRAM slot (SWDGE/gpsimd queue, rows stride W)
  - r_stage (one iteration later, sync queue): diagonal read stride W+1
    lands the shear -> bd fp16 (masked tail included)
  - score_stage: content matmuls + identity-add of bd (two concurrent
    64x64 diagonal PE tiles) accumulate per 512-col PSUM chunk; ScalarE
    exp straight from PSUM to fp16 (unnormalized); one chunked
    dma_start_transpose per head feeds the AV layout
  - av_stage (deferred one pair): vh carries a ones column, so row 64 of
    the AV output is the softmax denominator; reciprocal + K=1 broadcast
    matmul normalize on evacuation into concatT
  - output projection per pair right after head-pair 3's AV (overlapped)
"""

import os
import sys

sys.path.insert(0, "/opt/trn_rl_repo")

import numpy as np


import concourse.bass as bass
import concourse.mybir as mybir
from concourse.tile import TileContext, ScopedClock

F32 = mybir.dt.float32
F32R = mybir.dt.float32r
F16 = mybir.dt.float16
AF = mybir.ActivationFunctionType
OP = mybir.AluOpType

S, T, D, HC, DK, P = 1024, 2048, 1024, 8, 64, 128
DH = HC * DK  # 512, head-slice width per core
NQT = S // P  # 8 query tiles
WMAX = 2048 + 127  # max W (qi=7)
SLOT = P * (WMAX + 1) + 64  # dram scratch slot elements
EXP_BIAS = -7.0
NEG_BIG = -60000.0


def _patched_drain_and_barrier(self, tick_clock, wait_clock):
    # The walrus build in this container caps sync-waits per instruction;
    # Tile's stock tail drain carries one wait per live proc. Emit one SP nop
    # per wait instead, then the drain.
    dummy = mybir.InstNoOp(name="drain-wait-probe", ins=[], outs=[])
    dummy.engine = mybir.EngineType.SP
    wait_clock.add_sem_waits(dummy, ScopedClock({None: tick_clock.global_clock}))
    waits = []
    if dummy.sync_info is not None and dummy.sync_info.on_wait:
        waits = [(w.ant_name, w.wait_value) for w in dummy.sync_info.on_wait]
    assert self.sems is not None
    name2sem = {h.name: h for h in self.sems.allocated().values()}
    for name, val in waits:
        self.nc.sync.nop().wait_op(name2sem[name], val, "sem-ge")
    self.nc.sync.drain()
    self.nc.all_engine_barrier()
    popped = self.nc._tile_sem_poison_stack.pop()
    assert popped is self._sem_poison
    self.nc.clear_and_free_semaphores(list(self.sems.allocated().values()))
    self.nc.all_engine_barrier()


TileContext._drain_and_barrier = _patched_drain_and_barrier


def _split_multi_waits(nc, max_waits=1):
    """Walrus in this container rejects instructions carrying more than a
    couple of sync waits. Hoist extras onto same-engine NoOps just before
    the instruction (sequential on the engine, so semantics unchanged)."""
    for f in nc.m.functions:
        for bb in f.blocks:
            out = []
            changed = False
            for inst in bb.instructions:
                si = inst.sync_info
                if si is not None and si.on_wait and len(si.on_wait) > max_waits:
                    waits = list(si.on_wait)
                    for j, w in enumerate(waits[:-max_waits]):
                        nop = mybir.InstNoOp(
                            name=f"{inst.name}-wsplit{j}", ins=[], outs=[])
                        nop.engine = inst.engine
                        nop.sync_info = mybir.SyncInfo(on_wait=[w], on_update=[])
                        out.append(nop)
                    inst.sync_info = mybir.SyncInfo(
                        on_wait=waits[-max_waits:],
                        on_update=list(si.on_update))
                    changed = True
                out.append(inst)
            if changed:
                bb.instructions = out


def kq_of(qi):  # valid key count for query tile qi (keys j <= i + 1024)
    return (qi + 9) * P


def build_nc(split_waits=True):
    nc = bass.Bass(target_bir_lowering=True)

    qT = nc.declare_dram_parameter("qT", [D, S], F16, isOutput=False)
    kT = nc.declare_dram_parameter("kT", [D, T], F16, isOutput=False)
    vT = nc.declare_dram_parameter("vT", [D, T], F16, isOutput=False)
    RT = nc.declare_dram_parameter("RT", [D, T], F16, isOutput=False)
    Wq = nc.declare_dram_parameter("Wq", [D, DH], F16, isOutput=False)
    Wk = nc.declare_dram_parameter("Wk", [D, DH], F16, isOutput=False)
    Wv = nc.declare_dram_parameter("Wv", [D, DH], F16, isOutput=False)
    Wr = nc.declare_dram_parameter("Wr", [D, DH], F16, isOutput=False)
    Wo16 = nc.declare_dram_parameter("Wo16", [DH, D], F16, isOutput=False)
    ub = nc.declare_dram_parameter("ub", [P, 4], F32, isOutput=False)
    vb = nc.declare_dram_parameter("vb", [P, 4], F32, isOutput=False)
    mm2 = nc.declare_dram_parameter("mm2", [P, 3328], F16, isOutput=False)
    i128 = nc.declare_dram_parameter("i128", [P, P], F16, isOutput=False)
    ones_in = nc.declare_dram_parameter("ones_in", [1, DK], F32R, isOutput=False)
    outp = nc.declare_dram_parameter("out", [S, D], F32, isOutput=True)

    with TileContext(nc) as tc:
        with (
            tc.tile_pool(name="persist", bufs=1) as pp,
            tc.tile_pool(name="consts", bufs=1) as cp,
        ):
            # persistent fp16 tensors (partition = dk within head-pair tile)
            quT = pp.tile([P, 4 * S], F16)      # (qh+u).T   blocks hp
            qvT = pp.tile([P, 4 * S], F16)      # (qh+v).T
            khT = pp.tile([P, 4 * T], F16)
            rh2T = pp.tile([P, 4 * 3072], F16)
            vh16 = pp.tile([P, 16 * (HC * 65)], F16)  # key tile x 8 heads x 64+1
            concatT = pp.tile([P, 4 * S], F16)
            WoS = pp.tile([P, 4 * D], F16)
            ones1 = pp.tile([1, DK], F32R)
            nc.sync.dma_start(out=ones1[:], in_=ones_in[:])

            ub_sb = cp.tile([P, 4], F32)
            vb_sb = cp.tile([P, 4], F32)
            mm2_sb = cp.tile([P, 3328], F16)
            i128_sb = cp.tile([P, P], F16)
            expb_sb = cp.tile([P, 1], F32)
            nc.vector.memset(expb_sb[:], EXP_BIAS)

            nc.sync.dma_start(out=ub_sb[:], in_=ub[:])
            nc.sync.dma_start(out=vb_sb[:], in_=vb[:])
            nc.sync.dma_start(out=mm2_sb[:], in_=mm2[:])
            nc.sync.dma_start(out=i128_sb[:], in_=i128[:])
            # WoS layout [128, dt*1024 + o] <- Wo16[(dt p), o]
            for dt_ in range(4):
                nc.scalar.dma_start(
                    out=WoS[:, dt_ * D : (dt_ + 1) * D],
                    in_=Wo16[dt_ * P : (dt_ + 1) * P, :],
                )

            # ---------------- projections ----------------
            def load_w(pool, wparam, jit=False):
                wsb = pool.tile([P, 8 * DH], F16, tag="wsb")
                if not jit:
                    for kd in range(8):
                        nc.scalar.dma_start(
                            out=wsb[:, kd * DH : (kd + 1) * DH],
                            in_=wparam[kd * P : (kd + 1) * P, :],
                        )
                    return wsb
                return wsb, wparam

            # qhT-style projection: out[512, ncols] = W_s @ xT, evacuated by fn
            def proj_T(pool, psum, wsb, xparam, ncols, evac, wparam=None):
                nth = ncols // 1024
                for th in range(nth):
                    psums = {k: psum.tile([P, 512], F32, tag="proj", name="projps")
                             for k in [(d, t2) for d in range(4) for t2 in range(2)]}
                    for kd in range(8):
                        if wparam is not None and th == 0:
                            # JIT weight-chunk load: first matmul starts after
                            # one weight chunk instead of all eight
                            nc.scalar.dma_start(
                                out=wsb[:, kd * DH : (kd + 1) * DH],
                                in_=wparam[kd * P : (kd + 1) * P, :],
                            )
                        xsb = pool.tile([P, 1024], F16, tag="xstage")
                        nc.scalar.dma_start(
                            out=xsb[:],
                            in_=xparam[kd * P : (kd + 1) * P,
                                       th * 1024 : (th + 1) * 1024],
                        )
                        for dot in range(4):
                            for tc2 in range(2):
                                nc.tensor.matmul(
                                    psums[(dot, tc2)][:],
                                    wsb[:, kd * DH + dot * P : kd * DH + (dot + 1) * P],
                                    xsb[:, tc2 * 512 : (tc2 + 1) * 512],
                                    start=(kd == 0),
                                    stop=(kd == 7),
                                )
                    for dot in range(4):
                        for tc2 in range(2):
                            evac(psums[(dot, tc2)], dot, th * 1024 + tc2 * 512)

            with (
                tc.tile_pool(name="projp", bufs=3) as jp,
                tc.tile_pool(name="projw", bufs=2) as jw,
                tc.tile_pool(name="rhtmp", bufs=1) as jr,
                tc.tile_pool(name="projpsum", bufs=8, space="PSUM") as jps,
            ):
                wsb, _wp = load_w(jw, Wq, jit=True)

                def evac_q(ps, dot, col):
                    nc.vector.tensor_scalar(
                        quT[:, dot * S + col : dot * S + col + 512], ps[:],
                        ub_sb[:, dot : dot + 1], None, OP.add)
                    nc.vector.tensor_scalar(
                        qvT[:, dot * S + col : dot * S + col + 512], ps[:],
                        vb_sb[:, dot : dot + 1], None, OP.add)

                proj_T(jp, jps, wsb, qT, S, evac_q, wparam=_wp)

                # R before k: the attention m_stages need only qvT + rh2T,
                # so they can start overlapping while k/v still project
                rhT = jr.tile([P, 4 * T], F16, tag="rhT")
                wsb = load_w(jw, Wr)

                def evac_r(ps, dot, col):
                    nc.vector.tensor_copy(
                        rhT[:, dot * T + col : dot * T + col + 512], ps[:])

                proj_T(jp, jps, wsb, RT, T, evac_r)

                # rh2T[:, m'] = rhT[:, (m' + 1023) % 2048], m' in [0, 3072)
                for dot in range(4):
                    nc.vector.tensor_copy(
                        rh2T[:, dot * 3072 : dot * 3072 + 1025],
                        rhT[:, dot * T + 1023 : dot * T + 2048])
                    nc.vector.tensor_copy(
                        rh2T[:, dot * 3072 + 1025 : dot * 3072 + 3072],
                        rhT[:, dot * T : dot * T + 2047])

                wsb = load_w(jw, Wk)

                def evac_k(ps, dot, col):
                    nc.scalar.copy(
                        khT[:, dot * T + col : dot * T + col + 512], ps[:])

                proj_T(jp, jps, wsb, kT, T, evac_k)

                # vh (untransposed): per key tile tt, psum [128 keys, 512 dh]
                wsb = load_w(jw, Wv)
                for tg in range(2):
                    vps = {tl: jps.tile([P, 512], F32, tag="proj", name="vhps")
                           for tl in range(8)}
                    for kd in range(8):
                        vsb = jp.tile([P, 1024], F16, tag="xstage")
                        nc.scalar.dma_start(
                            out=vsb[:],
                            in_=vT[kd * P : (kd + 1) * P,
                                   tg * 1024 : (tg + 1) * 1024],
                        )
                        for tl in range(8):
                            nc.tensor.matmul(
                                vps[tl][:],
                                vsb[:, tl * P : (tl + 1) * P],
                                wsb[:, kd * DH : (kd + 1) * DH],
                                start=(kd == 0),
                                stop=(kd == 7),
                            )
                    for tl in range(8):
                        tt = tg * 8 + tl
                        base = tt * (HC * 65)
                        dst = bass.AP(vh16.tensor, vh16.offset + base,
                                      [[vh16.tensor.shape[1], P], [65, HC], [1, DK]])
                        nc.vector.tensor_copy(
                            dst, vps[tl][:].rearrange("p (h c) -> p h c", h=HC))
                        ones = bass.AP(vh16.tensor, vh16.offset + base + DK,
                                       [[vh16.tensor.shape[1], P], [65, HC]])
                        nc.vector.memset(ones, 1.0)

            # ---------------- attention ----------------
            with (
                tc.tile_pool(name="att_m", bufs=4) as mp,
                tc.tile_pool(name="att_bd", bufs=8) as bp,
                tc.tile_pool(name="att_att", bufs=4) as atp,
                tc.tile_pool(name="att_tr", bufs=4) as trp,
                tc.tile_pool(name="dram", bufs=10, space="DRAM") as dp,
                tc.tile_pool(name="ps_m", bufs=2, space="PSUM") as psm,
                tc.tile_pool(name="ps_ac", bufs=3, space="PSUM") as psac,
                tc.tile_pool(name="ps_o", bufs=1, space="PSUM") as pso,
                tc.tile_pool(name="ps_out", bufs=1, space="PSUM") as opso,
                tc.tile_pool(name="smalls", bufs=2) as smp,
            ):
                ITERS = [(hp, qi) for hp in range(4) for qi in range(NQT)]
                LA = 4  # M-stage lookahead (software pipeline depth)
                bd_tiles = {}
                mdr_slots = {}
                t_slots = {}
                atr_tiles = {}  # (hp, pair) -> [h0 tile, h1 tile]
                av_ready = []   # (hp, pair) queue; AV deferred by one pair

                def m_stage(idx):
                    hp, qi = ITERS[idx]
                    KQ = kq_of(qi)
                    W = KQ + 127
                    # position matrices M_h [128, W]; h0/h1 chunk-interleaved so
                    # the K=64 matmuls pack in PE row groups
                    msbs = [mp.tile([P, WMAX], F16, tag="msb",
                                    name=f"msb{idx}_{h}") for h in range(2)]
                    nwc = (W + 511) // 512
                    for wc in range(nwc):
                        nw = min(512, W - wc * 512)
                        mpss = [psm.tile([P, 512], F32, tag="mps",
                                         name=f"mps{idx}_{wc}_{h}")
                                for h in range(2)]
                        for h in range(2):
                            pr = slice(h * DK, (h + 1) * DK)
                            nc.tensor.matmul(
                                mpss[h][:, :nw],
                                qvT[pr, hp * S + qi * P : hp * S + (qi + 1) * P],
                                rh2T[pr, hp * 3072 + qi * P + wc * 512 :
                                     hp * 3072 + qi * P + wc * 512 + nw],
                                start=True, stop=True,
                            )
                        # evac + causal mask fused: masked cells get -big added
                        # via the precomputed mm2 plane (d = c - qi*128 + 1024)
                        moff = wc * 512 - qi * P + 1024
                        has_mask = moff + nw - 1 > 2048
                        for h in range(2):
                            dst = msbs[h][:, wc * 512 : wc * 512 + nw]
                            if has_mask:
                                # mask add only exists on DVE (STT)
                                nc.vector.scalar_tensor_tensor(
                                    dst, mpss[h][:, :nw], 1.0,
                                    mm2_sb[:, moff : moff + nw],
                                    OP.mult, OP.add)
                            elif h == 0:
                                nc.vector.tensor_copy(dst, mpss[h][:, :nw])
                            else:
                                nc.scalar.copy(dst, mpss[h][:, :nw])
                    # shear writes (SWDGE queue); rows stride W
                    for h in range(2):
                        mdr = dp.tile([SLOT], F16, tag="mscr",
                                      name=f"mdr{idx}_{h}")
                        nc.gpsimd.dma_start(
                            out=bass.AP(mdr.tensor, mdr.offset, [[W, P], [1, W]]),
                            in_=msbs[h][:, :W],
                        )
                        mdr_slots[(idx, h)] = (mdr, W, KQ)

                def r_stage(idx):
                    # shear reads, emitted one iteration after their write so
                    # the SWDGE stream never blocks on write completion
                    for h in range(2):
                        mdr, W, KQ = mdr_slots.pop((idx, h))
                        bd = bp.tile([P, T], F16, tag="bd", name=f"bd{idx}_{h}")
                        nc.sync.dma_start(
                            out=bd[:, :KQ],
                            in_=bass.AP(mdr.tensor, mdr.offset,
                                        [[W + 1, P], [1, KQ]]),
                        )
                        bd_tiles[(idx, h)] = bd

                def score_stage(idx):
                    hp, qi = ITERS[idx]
                    KQ = kq_of(qi)
                    njt = KQ // P
                    pair = qi // 2
                    if qi % 2 == 0:
                        KQ1 = kq_of(2 * pair + 1)
                        atr_tiles[(hp, pair)] = [
                            trp.tile([P, 256 * (KQ1 // P)], F16, tag="atr",
                                     name=f"atr_{hp}_{pair}_{hh}")
                            for hh in range(2)]
                    bds = [bd_tiles.pop((idx, h)) for h in range(2)]
                    atts = [atp.tile([P, T], F16, tag="att",
                                     name=f"att{idx}_{h}") for h in range(2)]
                    nkc = (KQ + 511) // 512
                    for kc in range(nkc):
                        off = kc * 512
                        nk = min(512, KQ - off)
                        acpss = [psac.tile([P, 512], F32, tag="acps",
                                           name=f"acps{idx}_{kc}_{h}")
                                 for h in range(2)]
                        for h in range(2):
                            pr = slice(h * DK, (h + 1) * DK)
                            nc.tensor.matmul(
                                acpss[h][:, :nk],
                                quT[pr, hp * S + qi * P : hp * S + (qi + 1) * P],
                                khT[pr, hp * T + off : hp * T + off + nk],
                                start=True, stop=False,
                            )
                        for h in range(2):
                            # identity-add as two concurrent diagonal
                            # 64x64 tiles (row+col groups)
                            for dg in range(2):
                                rp = slice(64 * dg, 64 * dg + 64)
                                nc.tensor.matmul(
                                    acpss[h][rp, :nk],
                                    i128_sb[rp, 64 * dg : 64 * dg + 64],
                                    bds[h][rp, off : off + nk],
                                    start=False, stop=(dg == 1),
                                    tile_position=(64 * dg, 64 * dg),
                                )
                        for h in range(2):
                            nc.scalar.activation(
                                atts[h][:, off : off + nk],
                                acpss[h][:, :nk], AF.Exp,
                                bias=expb_sb[:], scale=0.125)
                    # unnormalized att goes straight to the transpose; the
                    # softmax denominator rides the vh ones-row through AV
                    for h in range(2):
                        atr = atr_tiles[(hp, pair)][h]
                        nc.sync.dma_start_transpose(
                            out=bass.AP(
                                atr.tensor,
                                atr.offset + (qi % 2) * P,
                                [[atr.tensor.shape[1], P], [256, njt], [1, P]],
                            ),
                            in_=atts[h][:, :KQ],
                        )

                def av_stage(hp, pair):
                    # AV for the pair; 65-wide vh slices carry a ones column so
                    # row 64 of each head's output accumulates the softmax sums
                    KQ0, KQ1 = kq_of(2 * pair), kq_of(2 * pair + 1)
                    njt1 = KQ1 // P
                    tiles = atr_tiles.pop((hp, pair))
                    for h in range(2):
                        # zero-fill a0's missing key chunks
                        atr = tiles[h]
                        for jt in range(KQ0 // P, njt1):
                            nc.vector.memset(
                                atr[:, jt * 256 : jt * 256 + P], 0.0)
                    opss = []
                    for h in range(2):
                        ops = pso.tile([P, 256], F32, tag="ops",
                                       name=f"o{hp}_{pair}_{h}")
                        opss.append(ops)
                        atr = tiles[h]
                        for jt in range(njt1):
                            nc.tensor.matmul(
                                ops[0:65, :],
                                vh16[:, jt * (HC * 65) + (hp * 2 + h) * 65 :
                                     jt * (HC * 65) + (hp * 2 + h) * 65 + 65],
                                atr[:, jt * 256 : (jt + 1) * 256],
                                start=(jt == 0), stop=(jt == njt1 - 1),
                            )
                    for h in range(2):
                        ops = opss[h]
                        # normalize: recip of the sums row, broadcast via a
                        # K=1 matmul into the unused partitions 64-127 of the
                        # same psum tile, then scale the dh rows on evacuation
                        r_row = smp.tile([1, 256], F32R, tag="rrow",
                                         name=f"rr{hp}_{pair}_{h}")
                        with nc.allow_low_precision(
                                reason="f32r is bit-identical to f32"):
                            nc.vector.reciprocal(r_row[:], ops[64:65, :])
                        bps = pso.tile([P, 256], F32, tag="bps",
                                       name=f"bp{hp}_{pair}_{h}")
                        nc.tensor.matmul(
                            bps[0:DK, :], ones1[:], r_row[:],
                            start=True, stop=True)
                        rb = smp.tile([P, 256], F32, tag="rbsb",
                                      name=f"rb{hp}_{pair}_{h}")
                        nc.vector.tensor_copy(rb[0:DK, :], bps[0:DK, :])
                        nc.vector.tensor_tensor(
                            concatT[h * DK : (h + 1) * DK,
                                    hp * S + pair * 256 : hp * S + (pair + 1) * 256],
                            ops[0:DK, :], rb[0:DK, :], OP.mult)

                def outproj(pair):
                    # hp-major order means hp=3's AV is the last writer of this
                    # pair's concatT columns; project+store them immediately
                    for it in (2 * pair, 2 * pair + 1):
                        for oc in range(2):
                            ps = opso.tile([P, 512], F32, tag="out",
                                           name=f"ops{pair}_{it}_{oc}")
                            for dt in range(4):
                                nc.tensor.matmul(
                                    ps[:],
                                    concatT[:, dt * S + it * P : dt * S + (it + 1) * P],
                                    WoS[:, dt * D + oc * 512 : dt * D + (oc + 1) * 512],
                                    start=(dt == 0), stop=(dt == 3),
                                )
                            osb = smp.tile([P, 512], F32, tag="osb",
                                           name=f"osb{pair}_{it}_{oc}")
                            nc.vector.tensor_copy(osb[:], ps[:])
                            nc.sync.dma_start(
                                out=outp[it * P : (it + 1) * P,
                                         oc * 512 : (oc + 1) * 512],
                                in_=osb[:])

                def av_and_out(hp, pair):
                    av_stage(hp, pair)
                    if hp == 3:
                        outproj(pair)

                for idx in range(len(ITERS) + LA):
                    if idx < len(ITERS):
                        m_stage(idx)
                    if 1 <= idx <= len(ITERS):
                        r_stage(idx - 1)
                    cons = idx - LA
                    if cons >= 0:
                        score_stage(cons)
                        hp, qi = ITERS[cons]
                        if qi % 2 == 1:
                            av_ready.append((hp, qi // 2))
                            if len(av_ready) >= 2:
                                av_and_out(*av_ready.pop(0))
                for hp_pair in av_ready:
                    av_and_out(*hp_pair)

    if split_waits:
        _split_multi_waits(nc)
    return nc


def prep_core_inputs(core, q, k, v, u, v_bias, Wq, Wk, Wv, Wr, Wo, R):
    b, hh = core // 2, core % 2
    sl = slice(hh * DH, (hh + 1) * DH)
    c = np.ascontiguousarray
    f16 = np.float16
    ii = np.arange(P)
    dd = np.arange(3328)
    mm2 = np.where(dd[None, :] > 2 * ii[:, None] + 2048, NEG_BIG, 0.0).astype(f16)
    return {
        "qT": c(q[b].T).astype(f16),
        "kT": c(k[b].T).astype(f16),
        "vT": c(v[b].T).astype(f16),
        "RT": c(R.T).astype(f16),
        "Wq": c(Wq[sl, :].T).astype(f16),
        "Wk": c(Wk[sl, :].T).astype(f16),
        "Wv": c(Wv[sl, :].T).astype(f16),
        "Wr": c(Wr[sl, :].T).astype(f16),
        "Wo16": c(Wo[:, sl].T).astype(f16),
        "ub": c(u[0, hh * HC : (hh + 1) * HC, 0, :].reshape(4, P).T),
        "vb": c(v_bias[0, hh * HC : (hh + 1) * HC, 0, :].reshape(4, P).T),
        "mm2": mm2,
        "i128": np.eye(P, dtype=f16),
        "ones_in": np.ones((1, DK), np.float32),
    }


def combine_outputs(results):
    # results: list of 8 dicts with "out" [S, D]; partial sums per batch pair
    out = np.empty((4, S, D), np.float32)
    for b in range(4):
        out[b] = results[2 * b]["out"] + results[2 * b + 1]["out"]
    return out


_CACHED_NC = None
last_result = None  # BassKernelResults of the most recent run (for test harness)


def kernel(q, k, v, mask, u, v_bias, Wq, Wk, Wv, Wr, Wo, R):
    global _CACHED_NC, last_result
    from concourse.bass_utils import run_bass_kernel_spmd

    q, k, v = np.asarray(q), np.asarray(k), np.asarray(v)
    u, v_bias = np.asarray(u), np.asarray(v_bias)
    Wq, Wk, Wv, Wr, Wo, R = map(np.asarray, (Wq, Wk, Wv, Wr, Wo, R))

    # The kernel exploits the known TXL mask structure (j <= i + MEM).
    # Verify the passed mask matches; structural masking is baked in.
    m = np.asarray(mask)
    exp_mask = (np.arange(T)[None, :] <= np.arange(S)[:, None] + 1024)
    assert m.shape == (4, S, T) and bool((m == exp_mask[None]).all()), \
        "kernel compiled for the TXL causal mask (j <= i + MEM)"

    if _CACHED_NC is None:
        _CACHED_NC = build_nc()

    in_maps = [prep_core_inputs(c, q, k, v, u, v_bias, Wq, Wk, Wv, Wr, Wo, R)
               for c in range(8)]
    trace = bool(os.environ.get("TXL_TRACE"))
    kwargs = {}
    if trace:
        kwargs = {"trace": True, "tmpdir": os.environ.get("TXL_TRACE_DIR")}
    last_result = run_bass_kernel_spmd(_CACHED_NC, in_maps, list(range(8)), **kwargs)
    return combine_outputs(last_result.results)


# revision 77
# speedup vs baseline: 1.0211x; 1.0211x over previous
"""Transformer-XL relative multi-head attention, 8-way sharded on Trainium2.

Self-contained harness entry: kernel(**inputs) -> np.ndarray [4, 1024, 1024].

Sharding: core c handles batch b = c//2 and head-half hh = c%2 (8 of 16
heads). Each core computes a partial output (its heads' contribution
through Wo); the host unshard sums the two partials per batch (row-parallel
tensor parallelism for the output projection).

Pipeline, software-pipelined LA=4 iterations deep over (head-pair hp,
query tile qi), both heads h chunk-interleaved so K=64 matmuls pack in
PE row groups:
  - m_stage: position matmuls M_h [128, W] -> PSUM; evacuation to fp16
    fused with the causal mask (DVE scalar_tensor_tensor adding the
    precomputed mm2 plane; unmasked chunks split DVE/ACT); shear write
    to a DRAM slot (SWDGE/gpsimd queue, rows stride W)
  - r_stage (one iteration later, sync queue): diagonal read stride W+1
    lands the shear -> bd fp16 (masked tail included)
  - score_stage: content matmuls + identity-add of bd (two concurrent
    64x64 diagonal PE tiles) accumulate per 512-col PSUM chunk; ScalarE
    exp straight from PSUM to fp16 (unnormalized); one chunked
    dma_start_transpose per head feeds the AV layout
  - av_stage (deferred one pair): vh carries a ones column, so row 64 of
    the AV output is the softmax denominator; reciprocal + K=1 broadcast
    matmul normalize on evacuation into concatT
  - output projection per pair right after head-pair 3's AV (overlapped)
"""

import os
import sys

sys.path.insert(0, "/opt/trn_rl_repo")

import numpy as np


import concourse.bass as bass
import concourse.mybir as mybir
from concourse.tile import TileContext, ScopedClock

F32 = mybir.dt.float32
F32R = mybir.dt.float32r
F16 = mybir.dt.float16
AF = mybir.ActivationFunctionType
OP = mybir.AluOpType

S, T, D, HC, DK, P = 1024, 2048, 1024, 8, 64, 128
DH = HC * DK  # 512, head-slice width per core
NQT = S // P  # 8 query tiles
WMAX = 2048 + 127  # max W (qi=7)
SLOT = P * (WMAX + 1) + 64  # dram scratch slot elements
EXP_BIAS = -7.0
NEG_BIG = -60000.0


def _patched_drain_and_barrier(self, tick_clock, wait_clock):
    # The walrus build in this container caps sync-waits per instruction;
    # Tile's stock tail drain carries one wait per live proc. Emit one SP nop
    # per wait instead, then the drain.
    dummy = mybir.InstNoOp(name="drain-wait-probe", ins=[], outs=[])
    dummy.engine = mybir.EngineType.SP
    wait_clock.add_sem_waits(dummy, ScopedClock({None: tick_clock.global_clock}))
    waits = []
    if dummy.sync_info is not None and dummy.sync_info.on_wait:
        waits = [(w.ant_name, w.wait_value) for w in dummy.sync_info.on_wait]
    assert self.sems is not None
    name2sem = {h.name: h for h in self.sems.allocated().values()}
    for name, val in waits:
        self.nc.sync.nop().wait_op(name2sem[name], val, "sem-ge")
    self.nc.sync.drain()
    self.nc.all_engine_barrier()
    popped = self.nc._tile_sem_poison_stack.pop()
    assert popped is self._sem_poison
    self.nc.clear_and_free_semaphores(list(self.sems.allocated().values()))
    self.nc.all_engine_barrier()


TileContext._drain_and_barrier = _patched_drain_and_barrier


def _split_multi_waits(nc, max_waits=1):
    """Walrus in this container rejects instructions carrying more than a
    couple of sync waits. Hoist extras onto same-engine NoOps just before
    the instruction (sequential on the engine, so semantics unchanged)."""
    for f in nc.m.functions:
        for bb in f.blocks:
            out = []
            changed = False
            for inst in bb.instructions:
                si = inst.sync_info
                if si is not None and si.on_wait and len(si.on_wait) > max_waits:
                    waits = list(si.on_wait)
                    for j, w in enumerate(waits[:-max_waits]):
                        nop = mybir.InstNoOp(
                            name=f"{inst.name}-wsplit{j}", ins=[], outs=[])
                        nop.engine = inst.engine
                        nop.sync_info = mybir.SyncInfo(on_wait=[w], on_update=[])
                        out.append(nop)
                    inst.sync_info = mybir.SyncInfo(
                        on_wait=waits[-max_waits:],
                        on_update=list(si.on_update))
                    changed = True
                out.append(inst)
            if changed:
                bb.instructions = out


def kq_of(qi):  # valid key count for query tile qi (keys j <= i + 1024)
    return (qi + 9) * P


def build_nc(split_waits=True):
    nc = bass.Bass(target_bir_lowering=True)

    qT = nc.declare_dram_parameter("qT", [D, S], F16, isOutput=False)
    kT = nc.declare_dram_parameter("kT", [D, T], F16, isOutput=False)
    vT = nc.declare_dram_parameter("vT", [D, T], F16, isOutput=False)
    RT = nc.declare_dram_parameter("RT", [D, T], F16, isOutput=False)
    Wq = nc.declare_dram_parameter("Wq", [D, DH], F16, isOutput=False)
    Wk = nc.declare_dram_parameter("Wk", [D, DH], F16, isOutput=False)
    Wv = nc.declare_dram_parameter("Wv", [D, DH], F16, isOutput=False)
    Wr = nc.declare_dram_parameter("Wr", [D, DH], F16, isOutput=False)
    Wo16 = nc.declare_dram_parameter("Wo16", [DH, D], F16, isOutput=False)
    ub = nc.declare_dram_parameter("ub", [P, 4], F32, isOutput=False)
    vb = nc.declare_dram_parameter("vb", [P, 4], F32, isOutput=False)
    mm2 = nc.declare_dram_parameter("mm2", [P, 3328], F16, isOutput=False)
    i128 = nc.declare_dram_parameter("i128", [P, P], F16, isOutput=False)
    ones_in = nc.declare_dram_parameter("ones_in", [1, DK], F32R, isOutput=False)
    outp = nc.declare_dram_parameter("out", [S, D], F32, isOutput=True)

    with TileContext(nc) as tc:
        with (
            tc.tile_pool(name="persist", bufs=1) as pp,
            tc.tile_pool(name="consts", bufs=1) as cp,
        ):
            # persistent fp16 tensors (partition = dk within head-pair tile)
            quT = pp.tile([P, 4 * S], F16)      # (qh+u).T   blocks hp
            qvT = pp.tile([P, 4 * S], F16)      # (qh+v).T
            khT = pp.tile([P, 4 * T], F16)
            rh2T = pp.tile([P, 4 * 3072], F16)
            vh16 = pp.tile([P, 16 * (HC * 65)], F16)  # key tile x 8 heads x 64+1
            concatT = pp.tile([P, 4 * S], F16)
            WoS = pp.tile([P, 4 * D], F16)
            ones1 = pp.tile([1, DK], F32R)
            nc.sync.dma_start(out=ones1[:], in_=ones_in[:])

            ub_sb = cp.tile([P, 4], F32)
            vb_sb = cp.tile([P, 4], F32)
            mm2_sb = cp.tile([P, 3328], F16)
            i128_sb = cp.tile([P, P], F16)
            expb_sb = cp.tile([P, 1], F32)
            nc.vector.memset(expb_sb[:], EXP_BIAS)

            nc.sync.dma_start(out=ub_sb[:], in_=ub[:])
            nc.sync.dma_start(out=vb_sb[:], in_=vb[:])
            nc.sync.dma_start(out=mm2_sb[:], in_=mm2[:])
            nc.sync.dma_start(out=i128_sb[:], in_=i128[:])
            # WoS layout [128, dt*1024 + o] <- Wo16[(dt p), o]
            for dt_ in range(4):
                nc.scalar.dma_start(
                    out=WoS[:, dt_ * D : (dt_ + 1) * D],
                    in_=Wo16[dt_ * P : (dt_ + 1) * P, :],
                )

            # ---------------- projections ----------------
            def load_w(pool, wparam, jit=False):
                wsb = pool.tile([P, 8 * DH], F16, tag="wsb")
                if not jit:
                    for kd in range(8):
                        nc.scalar.dma_start(
                            out=wsb[:, kd * DH : (kd + 1) * DH],
                            in_=wparam[kd * P : (kd + 1) * P, :],
                        )
                    return wsb
                return wsb, wparam

            # qhT-style projection: out[512, ncols] = W_s @ xT, evacuated by fn
            def proj_T(pool, psum, wsb, xparam, ncols, evac, wparam=None):
                nth = ncols // 1024
                for th in range(nth):
                    psums = {k: psum.tile([P, 512], F32, tag="proj", name="projps")
                             for k in [(d, t2) for d in range(4) for t2 in range(2)]}
                    for kd in range(8):
                        if wparam is not None and th == 0:
                            # JIT weight-chunk load: first matmul starts after
                            # one weight chunk instead of all eight
                            nc.scalar.dma_start(
                                out=wsb[:, kd * DH : (kd + 1) * DH],
                                in_=wparam[kd * P : (kd + 1) * P, :],
                            )
                        xsb = pool.tile([P, 1024], F16, tag="xstage")
                        nc.scalar.dma_start(
                            out=xsb[:],
                            in_=xparam[kd * P : (kd + 1) * P,
                                       th * 1024 : (th + 1) * 1024],
                        )
                        for dot in range(4):
                            for tc2 in range(2):
                                nc.tensor.matmul(
                                    psums[(dot, tc2)][:],
                                    wsb[:, kd * DH + dot * P : kd * DH + (dot + 1) * P],
                                    xsb[:, tc2 * 512 : (tc2 + 1) * 512],
                                    start=(kd == 0),
                                    stop=(kd == 7),
                                )
                    for dot in range(4):
                        for tc2 in range(2):
                            evac(psums[(dot, tc2)], dot, th * 1024 + tc2 * 512)

            with (
                tc.tile_pool(name="projp", bufs=3) as jp,
                tc.tile_pool(name="projw", bufs=2) as jw,
                tc.tile_pool(name="rhtmp", bufs=1) as jr,
                tc.tile_pool(name="projpsum", bufs=8, space="PSUM") as jps,
            ):
                wsb, _wp = load_w(jw, Wq, jit=True)

                def evac_q(ps, dot, col):
                    nc.vector.tensor_scalar(
                        quT[:, dot * S + col : dot * S + col + 512], ps[:],
                        ub_sb[:, dot : dot + 1], None, OP.add)
                    nc.vector.tensor_scalar(
                        qvT[:, dot * S + col : dot * S + col + 512], ps[:],
                        vb_sb[:, dot : dot + 1], None, OP.add)

                proj_T(jp, jps, wsb, qT, S, evac_q, wparam=_wp)

                # R before k: the attention m_stages need only qvT + rh2T,
                # so they can start overlapping while k/v still project
                rhT = jr.tile([P, 4 * T], F16, tag="rhT")
                wsb = load_w(jw, Wr)

                def evac_r(ps, dot, col):
                    nc.vector.tensor_copy(
                        rhT[:, dot * T + col : dot * T + col + 512], ps[:])

                proj_T(jp, jps, wsb, RT, T, evac_r)

                # rh2T[:, m'] = rhT[:, (m' + 1023) % 2048], m' in [0, 3072)
                for dot in range(4):
                    nc.vector.tensor_copy(
                        rh2T[:, dot * 3072 : dot * 3072 + 1025],
                        rhT[:, dot * T + 1023 : dot * T + 2048])
                    nc.vector.tensor_copy(
                        rh2T[:, dot * 3072 + 1025 : dot * 3072 + 3072],
                        rhT[:, dot * T : dot * T + 2047])

                wsb = load_w(jw, Wk)

                def evac_k(ps, dot, col):
                    nc.scalar.copy(
                        khT[:, dot * T + col : dot * T + col + 512], ps[:])

                proj_T(jp, jps, wsb, kT, T, evac_k)

                # vh (untransposed): per key tile tt, psum [128 keys, 512 dh]
                wsb = load_w(jw, Wv)
                for tg in range(2):
                    vps = {tl: jps.tile([P, 512], F32, tag="proj", name="vhps")
                           for tl in range(8)}
                    for kd in range(8):
                        vsb = jp.tile([P, 1024], F16, tag="xstage")
                        nc.scalar.dma_start(
                            out=vsb[:],
                            in_=vT[kd * P : (kd + 1) * P,
                                   tg * 1024 : (tg + 1) * 1024],
                        )
                        for tl in range(8):
                            nc.tensor.matmul(
                                vps[tl][:],
                                vsb[:, tl * P : (tl + 1) * P],
                                wsb[:, kd * DH : (kd + 1) * DH],
                                start=(kd == 0),
                                stop=(kd == 7),
                            )
                    for tl in range(8):
                        tt = tg * 8 + tl
                        base = tt * (HC * 65)
                        dst = bass.AP(vh16.tensor, vh16.offset + base,
                                      [[vh16.tensor.shape[1], P], [65, HC], [1, DK]])
                        nc.vector.tensor_copy(
                            dst, vps[tl][:].rearrange("p (h c) -> p h c", h=HC))
                        ones = bass.AP(vh16.tensor, vh16.offset + base + DK,
                                       [[vh16.tensor.shape[1], P], [65, HC]])
                        nc.vector.memset(ones, 1.0)

            # ---------------- attention ----------------
            with (
                tc.tile_pool(name="att_m", bufs=4) as mp,
                tc.tile_pool(name="att_bd", bufs=8) as bp,
                tc.tile_pool(name="att_att", bufs=4) as atp,
                tc.tile_pool(name="att_tr", bufs=4) as trp,
                tc.tile_pool(name="dram", bufs=10, space="DRAM") as dp,
                tc.tile_pool(name="ps_m", bufs=2, space="PSUM") as psm,
                tc.tile_pool(name="ps_ac", bufs=3, space="PSUM") as psac,
                tc.tile_pool(name="ps_o", bufs=1, space="PSUM") as pso,
                tc.tile_pool(name="ps_out", bufs=1, space="PSUM") as opso,
                tc.tile_pool(name="smalls", bufs=2) as smp,
            ):
                ITERS = [(hp, qi) for hp in range(4) for qi in range(NQT)]
                LA = 4  # M-stage lookahead (software pipeline depth)
                bd_tiles = {}
                mdr_slots = {}
                t_slots = {}
                atr_tiles = {}  # (hp, pair) -> [h0 tile, h1 tile]
                av_ready = []   # (hp, pair) queue; AV deferred by one pair

                def m_stage(idx):
                    hp, qi = ITERS[idx]
                    KQ = kq_of(qi)
                    W = KQ + 127
                    # position matrices M_h [128, W]; h0/h1 chunk-interleaved so
                    # the K=64 matmuls pack in PE row groups
                    msbs = [mp.tile([P, WMAX], F16, tag="msb",
                                    name=f"msb{idx}_{h}") for h in range(2)]
                    nwc = (W + 511) // 512
                    for wc in range(nwc):
                        nw = min(512, W - wc * 512)
                        mpss = [psm.tile([P, 512], F32, tag="mps",
                                         name=f"mps{idx}_{wc}_{h}")
                                for h in range(2)]
                        for h in range(2):
                            pr = slice(h * DK, (h + 1) * DK)
                            nc.tensor.matmul(
                                mpss[h][:, :nw],
                                qvT[pr, hp * S + qi * P : hp * S + (qi + 1) * P],
                                rh2T[pr, hp * 3072 + qi * P + wc * 512 :
                                     hp * 3072 + qi * P + wc * 512 + nw],
                                start=True, stop=True,
                            )
                        # evac + causal mask fused: masked cells get -big added
                        # via the precomputed mm2 plane (d = c - qi*128 + 1024)
                        moff = wc * 512 - qi * P + 1024
                        has_mask = moff + nw - 1 > 2048
                        for h in range(2):
                            dst = msbs[h][:, wc * 512 : wc * 512 + nw]
                            if has_mask:
                                # mask add only exists on DVE (STT)
                                nc.vector.scalar_tensor_tensor(
                                    dst, mpss[h][:, :nw], 1.0,
                                    mm2_sb[:, moff : moff + nw],
                                    OP.mult, OP.add)
                            elif h == 0:
                                nc.vector.tensor_copy(dst, mpss[h][:, :nw])
                            else:
                                nc.scalar.copy(dst, mpss[h][:, :nw])
                    # shear writes (SWDGE queue); rows stride W
                    for h in range(2):
                        mdr = dp.tile([SLOT], F16, tag="mscr",
                                      name=f"mdr{idx}_{h}")
                        nc.gpsimd.dma_start(
                            out=bass.AP(mdr.tensor, mdr.offset, [[W, P], [1, W]]),
                            in_=msbs[h][:, :W],
                        )
                        mdr_slots[(idx, h)] = (mdr, W, KQ)

                def r_stage(idx):
                    # shear reads, emitted one iteration after their write so
                    # the SWDGE stream never blocks on write completion
                    for h in range(2):
                        mdr, W, KQ = mdr_slots.pop((idx, h))
                        bd = bp.tile([P, T], F16, tag="bd", name=f"bd{idx}_{h}")
                        nc.sync.dma_start(
                            out=bd[:, :KQ],
                            in_=bass.AP(mdr.tensor, mdr.offset,
                                        [[W + 1, P], [1, KQ]]),
                        )
                        bd_tiles[(idx, h)] = bd

                def score_stage(idx):
                    hp, qi = ITERS[idx]
                    KQ = kq_of(qi)
                    njt = KQ // P
                    pair = qi // 2
                    if qi % 2 == 0:
                        KQ1 = kq_of(2 * pair + 1)
                        atr_tiles[(hp, pair)] = [
                            trp.tile([P, 256 * (KQ1 // P)], F16, tag="atr",
                                     name=f"atr_{hp}_{pair}_{hh}")
                            for hh in range(2)]
                    bds = [bd_tiles.pop((idx, h)) for h in range(2)]
                    atts = [atp.tile([P, T], F16, tag="att",
                                     name=f"att{idx}_{h}") for h in range(2)]
                    nkc = (KQ + 511) // 512
                    for kc in range(nkc):
                        off = kc * 512
                        nk = min(512, KQ - off)
                        acpss = [psac.tile([P, 512], F32, tag="acps",
                                           name=f"acps{idx}_{kc}_{h}")
                                 for h in range(2)]
                        for h in range(2):
                            pr = slice(h * DK, (h + 1) * DK)
                            nc.tensor.matmul(
                                acpss[h][:, :nk],
                                quT[pr, hp * S + qi * P : hp * S + (qi + 1) * P],
                                khT[pr, hp * T + off : hp * T + off + nk],
                                start=True, stop=False,
                            )
                        for h in range(2):
                            # identity-add as two concurrent diagonal
                            # 64x64 tiles (row+col groups)
                            for dg in range(2):
                                rp = slice(64 * dg, 64 * dg + 64)
                                nc.tensor.matmul(
                                    acpss[h][rp, :nk],
                                    i128_sb[rp, 64 * dg : 64 * dg + 64],
                                    bds[h][rp, off : off + nk],
                                    start=False, stop=(dg == 1),
                                    tile_position=(64 * dg, 64 * dg),
                                )
                        for h in range(2):
                            nc.scalar.activation(
                                atts[h][:, off : off + nk],
                                acpss[h][:, :nk], AF.Exp,
                                bias=expb_sb[:], scale=0.125)
                    # unnormalized att goes straight to the transpose; the
                    # softmax denominator rides the vh ones-row through AV
                    for h in range(2):
                        atr = atr_tiles[(hp, pair)][h]
                        nc.sync.dma_start_transpose(
                            out=bass.AP(
                                atr.tensor,
                                atr.offset + (qi % 2) * P,
                                [[atr.tensor.shape[1], P], [256, njt], [1, P]],
                            ),
                            in_=atts[h][:, :KQ],
                        )

                def av_stage(hp, pair):
                    # AV for the pair; 65-wide vh slices carry a ones column so
                    # row 64 of each head's output accumulates the softmax sums
                    KQ0, KQ1 = kq_of(2 * pair), kq_of(2 * pair + 1)
                    njt1 = KQ1 // P
                    tiles = atr_tiles.pop((hp, pair))
                    for h in range(2):
                        # zero-fill a0's missing key chunks
                        atr = tiles[h]
                        for jt in range(KQ0 // P, njt1):
                            nc.vector.memset(
                                atr[:, jt * 256 : jt * 256 + P], 0.0)
                    opss = []
                    for h in range(2):
                        ops = pso.tile([P, 256], F32, tag="ops",
                                       name=f"o{hp}_{pair}_{h}")
                        opss.append(ops)
                        atr = tiles[h]
                        for jt in range(njt1):
                            nc.tensor.matmul(
                                ops[0:65, :],
                                vh16[:, jt * (HC * 65) + (hp * 2 + h) * 65 :
                                     jt * (HC * 65) + (hp * 2 + h) * 65 + 65],
                                atr[:, jt * 256 : (jt + 1) * 256],
                                start=(jt == 0), stop=(jt == njt1 - 1),
                            )
                    for h in range(2):
                        ops = opss[h]
                        # normalize: recip of the sums row, broadcast via a
                        # K=1 matmul into the unused partitions 64-127 of the
                        # same psum tile, then scale the dh rows on evacuation
                        r_row = smp.tile([1, 256], F32R, tag="rrow",
                                         name=f"rr{hp}_{pair}_{h}")
                        with nc.allow_low_precision(
                                reason="f32r is bit-identical to f32"):
                            nc.vector.reciprocal(r_row[:], ops[64:65, :])
                        bps = pso.tile([P, 256], F32, tag="bps",
                                       name=f"bp{hp}_{pair}_{h}")
                        nc.tensor.matmul(
                            bps[0:DK, :], ones1[:], r_row[:],
                            start=True, stop=True)
                        rb = smp.tile([P, 256], F32, tag="rbsb",
                                      name=f"rb{hp}_{pair}_{h}")
                        nc.vector.tensor_copy(rb[0:DK, :], bps[0:DK, :])
                        nc.vector.tensor_tensor(
                            concatT[h * DK : (h + 1) * DK,
                                    hp * S + pair * 256 : hp * S + (pair + 1) * 256],
                            ops[0:DK, :], rb[0:DK, :], OP.mult)

                def outproj(pair):
                    # hp-major order means hp=3's AV is the last writer of this
                    # pair's concatT columns; project+store them immediately
                    for it in (2 * pair, 2 * pair + 1):
                        for oc in range(2):
                            ps = opso.tile([P, 512], F32, tag="out",
                                           name=f"ops{pair}_{it}_{oc}")
                            for dt in range(4):
                                nc.tensor.matmul(
                                    ps[:],
                                    concatT[:, dt * S + it * P : dt * S + (it + 1) * P],
                                    WoS[:, dt * D + oc * 512 : dt * D + (oc + 1) * 512],
                                    start=(dt == 0), stop=(dt == 3),
                                )
                            osb = smp.tile([P, 512], F32, tag="osb",
                                           name=f"osb{pair}_{it}_{oc}")
                            nc.vector.tensor_copy(osb[:], ps[:])
                            nc.sync.dma_start(
                                out=outp[it * P : (it + 1) * P,
                                         oc * 512 : (oc + 1) * 512],
                                in_=osb[:])

                def av_and_out(hp, pair):
                    av_stage(hp, pair)
                    if hp == 3:
                        outproj(pair)

                for idx in range(len(ITERS) + LA):
                    if idx < len(ITERS):
                        m_stage(idx)
                    if 2 <= idx <= len(ITERS) + 1:
                        r_stage(idx - 2)
                    cons = idx - LA
                    if cons >= 0:
                        score_stage(cons)
                        hp, qi = ITERS[cons]
                        if qi % 2 == 1:
                            av_ready.append((hp, qi // 2))
                            if len(av_ready) >= 2:
                                av_and_out(*av_ready.pop(0))
                for hp_pair in av_ready:
                    av_and_out(*hp_pair)

    if split_waits:
        _split_multi_waits(nc)
    return nc


def prep_core_inputs(core, q, k, v, u, v_bias, Wq, Wk, Wv, Wr, Wo, R):
    b, hh = core // 2, core % 2
    sl = slice(hh * DH, (hh + 1) * DH)
    c = np.ascontiguousarray
    f16 = np.float16
    ii = np.arange(P)
    dd = np.arange(3328)
    mm2 = np.where(dd[None, :] > 2 * ii[:, None] + 2048, NEG_BIG, 0.0).astype(f16)
    return {
        "qT": c(q[b].T).astype(f16),
        "kT": c(k[b].T).astype(f16),
        "vT": c(v[b].T).astype(f16),
        "RT": c(R.T).astype(f16),
        "Wq": c(Wq[sl, :].T).astype(f16),
        "Wk": c(Wk[sl, :].T).astype(f16),
        "Wv": c(Wv[sl, :].T).astype(f16),
        "Wr": c(Wr[sl, :].T).astype(f16),
        "Wo16": c(Wo[:, sl].T).astype(f16),
        "ub": c(u[0, hh * HC : (hh + 1) * HC, 0, :].reshape(4, P).T),
        "vb": c(v_bias[0, hh * HC : (hh + 1) * HC, 0, :].reshape(4, P).T),
        "mm2": mm2,
        "i128": np.eye(P, dtype=f16),
        "ones_in": np.ones((1, DK), np.float32),
    }


def combine_outputs(results):
    # results: list of 8 dicts with "out" [S, D]; partial sums per batch pair
    out = np.empty((4, S, D), np.float32)
    for b in range(4):
        out[b] = results[2 * b]["out"] + results[2 * b + 1]["out"]
    return out


_CACHED_NC = None
last_result = None  # BassKernelResults of the most recent run (for test harness)


def kernel(q, k, v, mask, u, v_bias, Wq, Wk, Wv, Wr, Wo, R):
    global _CACHED_NC, last_result
    from concourse.bass_utils import run_bass_kernel_spmd

    q, k, v = np.asarray(q), np.asarray(k), np.asarray(v)
    u, v_bias = np.asarray(u), np.asarray(v_bias)
    Wq, Wk, Wv, Wr, Wo, R = map(np.asarray, (Wq, Wk, Wv, Wr, Wo, R))

    # The kernel exploits the known TXL mask structure (j <= i + MEM).
    # Verify the passed mask matches; structural masking is baked in.
    m = np.asarray(mask)
    exp_mask = (np.arange(T)[None, :] <= np.arange(S)[:, None] + 1024)
    assert m.shape == (4, S, T) and bool((m == exp_mask[None]).all()), \
        "kernel compiled for the TXL causal mask (j <= i + MEM)"

    if _CACHED_NC is None:
        _CACHED_NC = build_nc()

    in_maps = [prep_core_inputs(c, q, k, v, u, v_bias, Wq, Wk, Wv, Wr, Wo, R)
               for c in range(8)]
    trace = bool(os.environ.get("TXL_TRACE"))
    kwargs = {}
    if trace:
        kwargs = {"trace": True, "tmpdir": os.environ.get("TXL_TRACE_DIR")}
    last_result = run_bass_kernel_spmd(_CACHED_NC, in_maps, list(range(8)), **kwargs)
    return combine_outputs(last_result.results)
